# revision 34
# baseline (speedup 1.0000x reference)
"""TRN2 Bass kernel for nn_Actor (retrieval_knn).

Data-parallel over batch across 8 NeuronCores (8192 rows/core).
Per core: ap_gather embedding lookup (feature-major), MLP layer-1 on
TensorE fed straight from the gather groups (no x-assembly DMAs), then
scores vs the 2489-entry table with W2 absorbed into the table side
(scores = h @ (table@W2).T + table@b2).  The scores matmul uses a
3-term fp16 split (h1@G1 + h1@G2 + h2@G1 + c) which is fp32-grade
(0 argmax flips on the reference inputs) at fp16 streaming speed.
Per 128-row tile the PSUM scores are staged to SBUF by the Scalar
engine (freeing PSUM for the next tile) and DVE max8 + max_index give
the argmax.  Everything is fully unrolled - no hardware loops, no
back-edge barriers.
"""
import sys
sys.path.insert(0, '/opt/trn_rl_repo')
import numpy as np
import ml_dtypes

B = 65536
NCORES = 8
BC = B // NCORES            # 8192
NW, NPTAB, EMB = 1807, 2490, 10
NPROJ = NPTAB - 1           # 2489
NPROJP = 2496               # padded to a multiple of 4 halvings (pad = -30000)
HID = 40
NTILES = BC // 128          # 64
NIDX = BC // 8              # 1024 ids per gather group

_cache = {}


def _f16(x):
    return np.asarray(x, np.float32).astype(np.float16)


def _build(L=1):
    from concourse import bacc, mybir, bass
    from concourse.tile import TileContext
    import concourse.mybir as mb
    dt = mybir.dt
    nc = bacc.Bacc("TRN2", target_bir_lowering=False, debug=False, num_devices=NCORES)

    widx = nc.dram_tensor("widx", [128, 64], dt.int16, kind="ExternalInput")
    pidx = nc.dram_tensor("pidx", [128, 64], dt.int16, kind="ExternalInput")
    wtab16 = nc.dram_tensor("wtab16", [128, NW], dt.float32, kind="ExternalInput")
    ptab16 = nc.dram_tensor("ptab16", [128, NPTAB], dt.float32, kind="ExternalInput")
    w1t = nc.dram_tensor("w1t", [20, HID], dt.float32, kind="ExternalInput")
    b1e = nc.dram_tensor("b1e", [HID, 1], dt.float32, kind="ExternalInput")
    gstk = nc.dram_tensor("gstk", [122, NPROJP], dt.float16, kind="ExternalInput")
    out_ext = nc.dram_tensor("out", [128, NTILES * 8], dt.uint32, kind="ExternalOutput")

    with TileContext(nc) as tc:
        with tc.tile_pool(name="const", bufs=1) as cp, \
             tc.tile_pool(name="work", bufs=1) as wp, \
             tc.tile_pool(name="scb", bufs=4) as sp, \
             tc.tile_pool(name="hfp", bufs=2) as fp_, \
             tc.tile_pool(name="m8p", bufs=4) as mp, \
             tc.tile_pool(name="hps", bufs=1, space="PSUM") as hpp, \
             tc.tile_pool(name="scps", bufs=2, space="PSUM") as scp:
            t_wtab = cp.tile([128, NW], dt.float32)
            t_ptab = cp.tile([128, NPTAB], dt.float32)
            t_widx = cp.tile([128, 64], dt.int16)
            t_pidx = cp.tile([128, 64], dt.int16)
            t_w1t = cp.tile([20, HID], dt.float32)
            t_b1 = cp.tile([HID, 1], dt.float32)
            t_gstk = cp.tile([122, NPROJP], dt.float16)
            nc.sync.dma_start(out=t_wtab, in_=wtab16.ap())
            nc.sync.dma_start(out=t_ptab, in_=ptab16.ap())
            nc.sync.dma_start(out=t_widx, in_=widx.ap())
            nc.sync.dma_start(out=t_pidx, in_=pidx.ap())
            nc.sync.dma_start(out=t_w1t, in_=w1t.ap())
            nc.sync.dma_start(out=t_b1, in_=b1e.ap())
            nc.sync.dma_start(out=t_gstk, in_=gstk.ap())

            wg = wp.tile([128, NIDX], dt.float32)
            pg = wp.tile([128, NIDX], dt.float32)
            x = wp.tile([20, BC], dt.float32)
            # One h-stack tile per gather group so the scores phase of a
            # group can start while later groups' MLP still runs.
            # Rows 0-39 h1, 40-63 h1[0:24], 64-103 h2, 104-119 h1[24:40],
            # 120-121 ones (compute engines may only write at partition
            # bases 0/32/64/96; DMA fills the rest).
            hs = [wp.tile([122, NIDX], dt.float16, name=f"hs{g}", tag=f"hs{g}")
                  for g in range(8)]
            onesrow = wp.tile([2, NIDX], dt.float16)
            outbuf = wp.tile([128, NTILES * 8], dt.uint32)
            nc.vector.memset(onesrow, 1.0)
            for g in range(8):
                nc.sync.dma_start(out=hs[g][120:122, :], in_=onesrow)

            for _ in range(L):
                nc.gpsimd.ap_gather(out_ap=wg, in_ap=t_wtab, idxs_ap=t_widx,
                                    channels=128, num_elems=NW, d=1, num_idxs=NIDX)
                nc.gpsimd.ap_gather(out_ap=pg, in_ap=t_ptab, idxs_ap=t_pidx,
                                    channels=128, num_elems=NPTAB, d=1, num_idxs=NIDX)
                # MLP layer 1, one gather group (1024 rows) at a time:
                # h = relu(W1 @ [we; pe] + b1), written as fp16 h1,
                # duplicate h1, and fp16 residual h2 = h - h1.
                for g in range(8):
                    c0 = g * NIDX
                    # issue from ACT queue: SP is busy with input DMAs
                    nc.scalar.dma_start(out=x[0:10, c0:c0 + NIDX],
                                        in_=wg[16 * g:16 * g + 10, :])
                    nc.scalar.dma_start(out=x[10:20, c0:c0 + NIDX],
                                        in_=pg[16 * g:16 * g + 10, :])
                    hp = hpp.tile([HID, NIDX], dt.float32)
                    for k in (0, 512):
                        nc.tensor.matmul(hp[:, k:k + 512], lhsT=t_w1t,
                                         rhs=x[:, c0 + k:c0 + k + 512],
                                         start=True, stop=True)
                    nc.scalar.activation(hs[g][0:HID, :], hp,
                                         mb.ActivationFunctionType.Relu, bias=t_b1)
                    # h2 = relu(hp) - h1 off the critical DVE: ScalarE writes
                    # the f32 relu, GpSimd does the subtract (both idle here)
                    hf = fp_.tile([HID, NIDX], dt.float32, name="hf", tag="hf")
                    nc.scalar.activation(hf, hp,
                                         mb.ActivationFunctionType.Relu, bias=t_b1)
                    nc.gpsimd.tensor_sub(hs[g][64:104, :], hf, hs[g][0:HID, :])
                    nc.sync.dma_start(out=hs[g][40:64, :], in_=hs[g][0:24, :])
                    nc.sync.dma_start(out=hs[g][104:120, :], in_=hs[g][24:40, :])

                # scores + argmax, one 128-row tile at a time.
                # GpSimd pre-compacts the row by pairwise max (2496 -> 1248
                # -> 624) so DVE's Max pass is short; MaxIndex still scans
                # the full row (exact fp32, identical result).
                for t in range(NTILES):
                    g, j = t // 8, t % 8
                    lhsT = hs[g][:, j * 128:(j + 1) * 128]
                    scb = sp.tile([128, NPROJP], dt.float32)
                    # two PSUM half-tiles (3 banks each) ping-pong so PE can
                    # fill one half while ScalarE drains the other to SBUF
                    H = NPROJP // 2
                    for half in range(2):
                        sc = scp.tile([128, H], dt.float32, name="sc", tag="sc")
                        for s0 in range(0, H, 512):
                            sw = min(512, H - s0)
                            nc.tensor.matmul(sc[:, s0:s0 + sw], lhsT=lhsT,
                                             rhs=t_gstk[:, half * H + s0:half * H + s0 + sw],
                                             start=True, stop=True)
                        nc.scalar.activation(scb[:, half * H:(half + 1) * H], sc,
                                             mb.ActivationFunctionType.Copy)
                    m8 = mp.tile([128, 8], dt.float32)
                    nc.vector.max(out=m8, in_=scb)
                    nc.vector.max_index(out=outbuf[:, t * 8:(t + 1) * 8],
                                        in_max=m8, in_values=scb)

            nc.sync.dma_start(out=out_ext.ap(), in_=outbuf)
    nc.compile()
    return nc


def _host_prep(inputs):
    worker_ids = np.asarray(inputs["worker_ids"]).astype(np.int64)
    project_ids = np.asarray(inputs["project_ids"]).astype(np.int64)
    worker_emb = np.asarray(inputs["worker_emb"], dtype=np.float32)
    project_emb = np.asarray(inputs["project_emb"], dtype=np.float32)
    W1 = np.asarray(inputs["W1"], dtype=np.float32)
    b1 = np.asarray(inputs["b1"], dtype=np.float32)
    W2 = np.asarray(inputs["W2"], dtype=np.float32)
    b2 = np.asarray(inputs["b2"], dtype=np.float32)

    table = project_emb[1:]
    G = (table @ W2).astype(np.float32)
    c = (table @ b2).astype(np.float32)
    G1 = _f16(G)
    G2 = _f16(G - G1.astype(np.float32))
    c1 = _f16(c)
    c2 = _f16(c - c1.astype(np.float32))
    gstk = np.zeros((122, NPROJP), dtype=np.float16)
    gstk[0:40, 0:NPROJ] = G1.T
    gstk[40:64, 0:NPROJ] = G2.T[0:24]
    gstk[64:104, 0:NPROJ] = G1.T
    gstk[104:120, 0:NPROJ] = G2.T[24:40]
    gstk[120, 0:NPROJ] = c1
    gstk[121, 0:NPROJ] = c2
    gstk[120, NPROJ:] = -30000.0   # pad columns can never win the argmax

    def gtab16(emb, nrow):
        t = np.zeros((16, nrow), dtype=np.float32)
        t[0:EMB] = emb.T
        return np.tile(t, (8, 1))          # pre-replicated for all 8 groups

    def idx_layout(ids_core):
        # [8 groups, 64 slots, 16 parts] -> [8, 16, 64] -> [128, 64]
        return ids_core.astype(np.int16).reshape(8, 64, 16).transpose(0, 2, 1).reshape(128, 64)

    shared = {
        "wtab16": gtab16(worker_emb, NW), "ptab16": gtab16(project_emb, NPTAB),
        "w1t": W1.T.astype(np.float32).copy(),
        "b1e": b1.reshape(HID, 1).astype(np.float32),
        "gstk": gstk,
    }
    in_maps = []
    for core in range(NCORES):
        sl = slice(core * BC, (core + 1) * BC)
        m = dict(shared)
        m["widx"] = idx_layout(worker_ids[sl])
        m["pidx"] = idx_layout(project_ids[sl])
        in_maps.append(m)
    return in_maps


def _decode(results):
    idx = np.zeros((B,), dtype=np.int64)
    for core in range(NCORES):
        o = results[core]["out"]          # [128, 8*NTILES] uint32
        for t in range(NTILES):
            rows = slice(core * BC + t * 128, core * BC + (t + 1) * 128)
            idx[rows] = o[:, 8 * t]
    return (idx + 1).astype(np.int32).reshape(B, 1)


def kernel(**inputs):
    from concourse.bass_utils import run_bass_kernel_spmd
    in_maps = _host_prep(inputs)
    if "nc1" not in _cache:
        _cache["nc1"] = _build(L=1)
    res = run_bass_kernel_spmd(_cache["nc1"], in_maps, core_ids=list(range(NCORES)))
    return _decode(res.results)
